# revision 18
# baseline (speedup 1.0000x reference)
"""Trainium2 Bass kernel for a GAT block.

Math (after algebraic simplification of the reference):
  h[b,f,n,k] = x[b,:,f,n] @ W[:,k] + bW[k]
  s2[b,f,n]  = h[b,f,n,:] @ a2 = v.x  (+c0 and s1/ab cancel inside softmax)
  d[b,f,n]   = softmax_n(s2)[n] * mask[n,n]
  out[b,k,f,n] = d[b,f,n] * h[b,f,n,k] = sum_c W[c,k] (x*d)[c,f,n] + bW[k] d[f,n]

Sharding: data-parallel over batch, 4 batches per core on 8 cores.

fp16 data path end to end (inputs converted and pre-transposed on host,
outputs converted back on host); all PE matmuls run at 1 cycle/row.

Per-batch input: xr [96, 4, 400] fp16 (row = 32c+fsub, cols (q, f', n),
frame = 16 fsub + f') loaded once into x4d [128, 1600] (rows 96:128
memset to 1.0 for the bias path; 3.2KB DMA descriptors).

Per (batch, 512-frame q-unit) on x4 = x4d[:, 400q:400q+400]:
  front-end: ps_s [32,400] = W2.T @ x4[0:96] (PE; scores in x4 layout),
    exp (ACT), reduce_sum + reciprocal (DVE, fp16), e*md and *1/z (GPS)
    -> dd [32,400] fp16
  MM2: pdd [128,400] = rep4.T @ dd (PE broadcast into 4 c-blocks)
  x4s [128,400] fp16 = x4 * pdd (DVE)
  8 matmul PAIRS into 2-bank psum tiles [128,2,512]; ONE eviction per
  pair (amortizes PSUM access latency), DVE pairs {0,2,4,6} / ACT
  {1,3,5,7}; 2 half stores per unit ([128,3200] fp16, 6.4KB
  descriptors), issued from the Sync sequencer.

PSUM is ONE ring of 4 two-bank slots (8 banks): per unit it carries
[pair0..pair6, front, pair7] where the front slot holds pdd of unit
u+1 (bank 0) and the scores of unit u+2 (bank 1).

Emission order is READINESS order per engine (engines dispatch
in-order; a not-yet-ready op at the queue head blocks later ready
ops).  Unit u's iteration interleaves: pairs 0-7 of u; exp/reduce/
recip/em/dd of u+1 staggered between pairs; and at the tail (between
pair 6 and 7) MM2+x4s of u+1 and MM1 of u+2 so the PE stream never
breaks at unit boundaries (PE p-state ramps to max only after ~3us of
continuous execution, doubling matmul speed).
"""

import sys

if "/opt/trn_rl_repo" not in sys.path:
    sys.path.insert(0, "/opt/trn_rl_repo")

import numpy as np

B, C, F, N, H = 32, 3, 2048, 25, 64
NCORES = 8
BPC = B // NCORES   # batches per core
QF = 512            # frames per q-unit
NQ = F // QF        # q-units per batch
FSUB = 16           # frames per fsub row
NS = QF // FSUB     # 32 fsub rows per q-unit
FN = F * N
TW = FSUB * N       # 400, columns per tile
NT = NS // 2        # 16 tiles (of 32 frames) per q-unit
NPAIR = NT // 2     # 8 psum pairs per q-unit
BANK = 512          # psum bank, f32 elems per partition

_NC_CACHE = {}


def _build_nc():
    import concourse.bass as bass
    import concourse.bacc as bacc
    import concourse.tile as tile
    from concourse import mybir

    f32 = mybir.dt.float32
    f16 = mybir.dt.float16
    MULT = mybir.AluOpType.mult
    AX = mybir.AxisListType.X
    EXP = mybir.ActivationFunctionType.Exp

    nc = bacc.Bacc()
    x_d = nc.declare_dram_parameter("xr", [BPC, 96, NQ * TW], f16, isOutput=False)
    wsel_d = nc.declare_dram_parameter("wsel", [128, NT, 128], f16, isOutput=False)
    rep4_d = nc.declare_dram_parameter("rep4", [NS, 128], f16, isOutput=False)
    xv_d = nc.declare_dram_parameter("xv", [BPC, C, NS, NQ * TW], f16, isOutput=False)
    md_d = nc.declare_dram_parameter("md16", [NS, TW], f16, isOutput=False)
    out_d = nc.declare_dram_parameter("out", [BPC, H, F, N], f16, isOutput=True)

    with tile.TileContext(nc) as tc:
        with (
            tc.tile_pool(name="singles", bufs=1) as singles,
            tc.tile_pool(name="x4d", bufs=3) as x4d_pool,
            tc.tile_pool(name="x4s", bufs=3) as x4s_pool,
            tc.tile_pool(name="sm", bufs=4) as sm_pool,
            tc.tile_pool(name="s2", bufs=3) as s2_pool,
            tc.tile_pool(name="osb", bufs=3) as osb_pool,
            tc.tile_pool(name="ps", bufs=4, space="PSUM") as ps_pool,
        ):
            wsel_sb = singles.tile([128, NT, 128], f16)
            nc.sync.dma_start(out=wsel_sb[:], in_=wsel_d[:, :, :])
            rep4_sb = singles.tile([NS, 128], f16)
            nc.sync.dma_start(out=rep4_sb[:], in_=rep4_d[:, :])
            md_sb = singles.tile([NS, TW], f16)
            nc.sync.dma_start(out=md_sb[:], in_=md_d[:, :])

            ADD_OP = mybir.AluOpType.add

            def load_batch(b):
                """Load one batch's x into a [128, 1600] x4d tile, and
                accumulate the three pre-scaled c-blocks of xv into the
                batch's scores tile s2 [32, 1600] via CCE add DMAs."""
                x4d = x4d_pool.tile([128, NQ * TW], f16, tag="x4d")
                nc.gpsimd.memset(x4d[96:128, :], 1.0)
                nc.sync.dma_start(out=x4d[0:96, :], in_=x_d[b, :, :])
                s2 = s2_pool.tile([NS, NQ * TW], f16, tag="s2")
                nc.sync.dma_start(out=s2[:], in_=xv_d[b, 0, :, :])
                nc.gpsimd.dma_start(out=s2[:], in_=xv_d[b, 1, :, :], accum_op=ADD_OP)
                nc.gpsimd.dma_start(out=s2[:], in_=xv_d[b, 2, :, :], accum_op=ADD_OP)
                return (x4d, s2)

            units = [(b, q) for b in range(BPC) for q in range(NQ)]
            nu = len(units)

            x4d_tiles = [None] * BPC
            x4d_tiles[0] = load_batch(0)
            if BPC > 1:
                x4d_tiles[1] = load_batch(1)

            def x4_view(ui):
                b, q = units[ui]
                return x4d_tiles[b][0][:, q * TW : (q + 1) * TW]

            def s2_view(ui):
                b, q = units[ui]
                return x4d_tiles[b][1][:, q * TW : (q + 1) * TW]

            def new_front():
                return ps_pool.tile([128, 2, BANK], f32, tag="ph2", name="front")

            def emit_exp(ui):
                e = sm_pool.tile([NS, TW], f16, tag="e")
                nc.scalar.activation(out=e[:], in_=s2_view(ui), func=EXP)
                return e

            def emit_zr(e):
                ev = e[:].rearrange("p (a b) -> p a b", b=N)
                z = sm_pool.tile([NS, FSUB], f16, tag="z")
                nc.vector.reduce_sum(out=z[:], in_=ev, axis=AX)
                r = sm_pool.tile([NS, FSUB], f16, tag="r")
                nc.vector.reciprocal(out=r[:], in_=z[:])
                return r

            def emit_em(e):
                em = sm_pool.tile([NS, TW], f16, tag="em")
                nc.gpsimd.tensor_tensor(out=em[:], in0=e[:], in1=md_sb[:], op=MULT)
                return em

            def emit_dd(em, r):
                dd = sm_pool.tile([NS, TW], f16, tag="dd")
                rr = r[:, :]
                r_bc = bass.AP(
                    tensor=rr.tensor,
                    offset=rr.offset,
                    ap=[rr.ap[0], [1, FSUB], [0, N]],
                )
                nc.gpsimd.tensor_tensor(out=dd[:], in0=em[:], in1=r_bc, op=MULT)
                return dd

            def emit_mm2_x4s(front, dd, ui):
                """pdd into front's bank 0, then x4s = x4 * pdd."""
                pdd = front[:, 0, 0:TW]
                nc.tensor.matmul(pdd, rep4_sb[:], dd[:], start=True, stop=True)
                x4s = x4s_pool.tile([128, TW], f16, tag="x4s")
                nc.vector.tensor_tensor(out=x4s[:], in0=x4_view(ui), in1=pdd, op=MULT)
                return x4s

            # ---- prologue: front-end chains for units 0 and 1
            with nc.allow_low_precision("fp16 softmax tolerates 1e-3"):
                def chain(ui):
                    e = emit_exp(ui)
                    r = emit_zr(e)
                    em = emit_em(e)
                    return emit_dd(em, r)

                dd_by_ui = {}
                dd_by_ui[0] = chain(0)
                if nu > 1:
                    dd_by_ui[1] = chain(1)
                front_cur = new_front()   # pdd_0
                x4s_cur = emit_mm2_x4s(front_cur, dd_by_ui.pop(0), 0)

                for ui, u in enumerate(units):
                    b, q = u
                    f0 = q * QF
                    have_next = ui + 1 < nu
                    x4s_u = x4s_cur
                    osb = osb_pool.tile([128, NT, TW], f16)

                    def pair(pr):
                        ph2 = ps_pool.tile(
                            [128, 2, BANK], f32, tag="ph2", name="ph2"
                        )
                        for i in (0, 1):
                            nc.tensor.matmul(
                                ph2[:, i, 0:TW],
                                wsel_sb[:, 2 * pr + i, :],
                                x4s_u[:, :],
                                start=True,
                                stop=True,
                            )
                        src = ph2[:, :, 0:TW]
                        dst = osb[:, 2 * pr : 2 * pr + 2, :]
                        dve = (pr % 2 == 0) if ui % 2 == 0 else (pr in (1, 3, 5))
                        if dve:
                            nc.vector.tensor_copy(dst, src)
                        else:
                            nc.scalar.copy(dst, src)

                    def store(hh):
                        osl = out_d[b, :, f0 : f0 + 1, :]
                        dmadst = bass.AP(
                            tensor=osl.tensor,
                            offset=osl.offset + hh * 8 * TW,
                            ap=[[FN, H], [16 * TW, 2], [1, 8 * TW]],
                        )
                        eng = nc.sync if hh == 0 else nc.gpsimd
                        eng.dma_start(
                            out=dmadst,
                            in_=osb[:, 8 * hh : 8 * (hh + 1), :],
                        )

                    have_far = ui + 2 < nu
                    if have_far:
                        e_n = emit_exp(ui + 2)
                    pair(0)
                    pair(1)
                    pair(2)
                    pair(3)
                    store(0)
                    if have_far:
                        r_n = emit_zr(e_n)
                        em_n = emit_em(e_n)
                    pair(4)
                    pair(5)
                    if have_far:
                        dd_by_ui[ui + 2] = emit_dd(em_n, r_n)
                    pair(6)
                    if have_next:
                        front_nxt = new_front()  # pdd_{u+1}
                        x4s_cur = emit_mm2_x4s(front_nxt, dd_by_ui.pop(ui + 1), ui + 1)
                        front_cur = front_nxt
                    pair(7)
                    store(1)
                    # prefetch the next batch near the end of each batch
                    if q == NQ - 1 and b + 2 < BPC and x4d_tiles[b + 2] is None:
                        x4d_tiles[b + 2] = load_batch(b + 2)
    nc.compile()
    return nc


def _get_nc():
    if "nc" not in _NC_CACHE:
        _NC_CACHE["nc"] = _build_nc()
    return _NC_CACHE["nc"]


def _make_in_maps(x, mask, W, bW, a1, a2, ab):
    x16 = np.asarray(x, np.float32).astype(np.float16)
    mask = np.asarray(mask, np.float32)
    W = np.asarray(W, np.float32)
    bW = np.asarray(bW, np.float32)
    a2 = np.asarray(a2, np.float32)

    # xr[b, 32c+fsub, (q, f', n)] = x[b, c, 512q + 16 fsub + f', n]
    xr = np.ascontiguousarray(
        x16.reshape(B, C, NQ, NS, FSUB, N)
        .transpose(0, 1, 3, 2, 4, 5)
        .reshape(B, C * NS, NQ * TW)
    )

    v = (W @ a2).astype(np.float32)                    # [C]
    md = np.diag(mask).astype(np.float16)              # [N]

    # xv[b, c, fsub, (q, f', n)] = v_c * x[b, c, 512q + 16 fsub + f', n]
    xf = np.asarray(x, np.float32)
    xv = np.ascontiguousarray(
        (v[None, :, None] * xf.reshape(B, C, F * N))
        .reshape(B, C, NQ, NS, FSUB, N)
        .transpose(0, 1, 3, 2, 4, 5)
        .reshape(B, C, NS, NQ * TW)
        .astype(np.float16)
    )

    # wsel[row = 32 c + fsub, tp, col = 2 k + jj]:
    #   delta[fsub == tp + 16 jj] * (W[c, k] if c < 3 else bW[k])
    # (column order (k, jj)-interleaved so the store DMA is affine)
    wsel = np.zeros((128, NT, 128), np.float16)
    cols = np.arange(H)
    W16 = W.astype(np.float16)
    bW16 = bW.astype(np.float16)
    for tp in range(NT):
        for jj in range(2):
            fsub = tp + 16 * jj
            for c in range(C):
                wsel[32 * c + fsub, tp, 2 * cols + jj] = W16[c]
            wsel[96 + fsub, tp, 2 * cols + jj] = bW16
    rep4 = np.zeros((NS, 128), np.float16)
    for blk in range(4):
        rep4[:, 32 * blk : 32 * (blk + 1)] = np.eye(NS, dtype=np.float16)
    md16 = np.tile(np.tile(md, FSUB)[None, :], (NS, 1)).astype(np.float16)

    in_maps = []
    for cix in range(NCORES):
        in_maps.append(
            {
                "xr": np.ascontiguousarray(xr[cix * BPC : (cix + 1) * BPC]),
                "wsel": wsel,
                "rep4": rep4,
                "xv": np.ascontiguousarray(xv[cix * BPC : (cix + 1) * BPC]),
                "md16": md16,
            }
        )
    return in_maps


def run(x, mask, W, bW, a1, a2, ab, **run_kwargs):
    from concourse.bass_utils import run_bass_kernel_spmd

    nc = _get_nc()
    in_maps = _make_in_maps(x, mask, W, bW, a1, a2, ab)
    res = run_bass_kernel_spmd(nc, in_maps, core_ids=list(range(NCORES)), **run_kwargs)
    out = np.concatenate(
        [res.results[i]["out"] for i in range(NCORES)], axis=0
    ).astype(np.float32)
    return out, res


def kernel(x, mask, W, bW, a1, a2, ab):
    out, _ = run(x, mask, W, bW, a1, a2, ab)
    return out


# revision 19
# speedup vs baseline: 1.0221x; 1.0221x over previous
"""Trainium2 Bass kernel for a GAT block.

Math (after algebraic simplification of the reference):
  h[b,f,n,k] = x[b,:,f,n] @ W[:,k] + bW[k]
  s2[b,f,n]  = h[b,f,n,:] @ a2 = v.x  (+c0 and s1/ab cancel inside softmax)
  d[b,f,n]   = softmax_n(s2)[n] * mask[n,n]
  out[b,k,f,n] = d[b,f,n] * h[b,f,n,k] = sum_c W[c,k] (x*d)[c,f,n] + bW[k] d[f,n]

Sharding: data-parallel over batch, 4 batches per core on 8 cores.

fp16 data path end to end (inputs converted and pre-transposed on host,
outputs converted back on host); all PE matmuls run at 1 cycle/row.

Per-batch input: xr [96, 4, 400] fp16 (row = 32c+fsub, cols (q, f', n),
frame = 16 fsub + f') loaded once into x4d [128, 1600] (rows 96:128
memset to 1.0 for the bias path; 3.2KB DMA descriptors).

Per (batch, 512-frame q-unit) on x4 = x4d[:, 400q:400q+400]:
  front-end: ps_s [32,400] = W2.T @ x4[0:96] (PE; scores in x4 layout),
    exp (ACT), reduce_sum + reciprocal (DVE, fp16), e*md and *1/z (GPS)
    -> dd [32,400] fp16
  MM2: pdd [128,400] = rep4.T @ dd (PE broadcast into 4 c-blocks)
  x4s [128,400] fp16 = x4 * pdd (DVE)
  8 matmul PAIRS into 2-bank psum tiles [128,2,512]; ONE eviction per
  pair (amortizes PSUM access latency), DVE pairs {0,2,4,6} / ACT
  {1,3,5,7}; 2 half stores per unit ([128,3200] fp16, 6.4KB
  descriptors), issued from the Sync sequencer.

PSUM is ONE ring of 4 two-bank slots (8 banks): per unit it carries
[pair0..pair6, front, pair7] where the front slot holds pdd of unit
u+1 (bank 0) and the scores of unit u+2 (bank 1).

Emission order is READINESS order per engine (engines dispatch
in-order; a not-yet-ready op at the queue head blocks later ready
ops).  Unit u's iteration interleaves: pairs 0-7 of u; exp/reduce/
recip/em/dd of u+1 staggered between pairs; and at the tail (between
pair 6 and 7) MM2+x4s of u+1 and MM1 of u+2 so the PE stream never
breaks at unit boundaries (PE p-state ramps to max only after ~3us of
continuous execution, doubling matmul speed).
"""

import sys

if "/opt/trn_rl_repo" not in sys.path:
    sys.path.insert(0, "/opt/trn_rl_repo")

import numpy as np

B, C, F, N, H = 32, 3, 2048, 25, 64
NCORES = 8
BPC = B // NCORES   # batches per core
QF = 512            # frames per q-unit
NQ = F // QF        # q-units per batch
FSUB = 16           # frames per fsub row
NS = QF // FSUB     # 32 fsub rows per q-unit
FN = F * N
TW = FSUB * N       # 400, columns per tile
NT = NS // 2        # 16 tiles (of 32 frames) per q-unit
NPAIR = NT // 2     # 8 psum pairs per q-unit
BANK = 512          # psum bank, f32 elems per partition

_NC_CACHE = {}


def _build_nc():
    import concourse.bass as bass
    import concourse.bacc as bacc
    import concourse.tile as tile
    from concourse import mybir

    f32 = mybir.dt.float32
    f16 = mybir.dt.float16
    MULT = mybir.AluOpType.mult
    AX = mybir.AxisListType.X
    EXP = mybir.ActivationFunctionType.Exp

    nc = bacc.Bacc()
    x_d = nc.declare_dram_parameter("xr", [BPC, 96, NQ * TW], f16, isOutput=False)
    wsel_d = nc.declare_dram_parameter("wsel", [128, NT, 128], f16, isOutput=False)
    rep4_d = nc.declare_dram_parameter("rep4", [NS, 128], f16, isOutput=False)
    xv_d = nc.declare_dram_parameter("xv", [BPC, C, NS, NQ * TW], f16, isOutput=False)
    md_d = nc.declare_dram_parameter("md16", [NS, TW], f16, isOutput=False)
    out_d = nc.declare_dram_parameter("out", [BPC, H, F, N], f16, isOutput=True)

    with tile.TileContext(nc) as tc:
        with (
            tc.tile_pool(name="singles", bufs=1) as singles,
            tc.tile_pool(name="x4d", bufs=3) as x4d_pool,
            tc.tile_pool(name="x4s", bufs=3) as x4s_pool,
            tc.tile_pool(name="sm", bufs=4) as sm_pool,
            tc.tile_pool(name="s2", bufs=3) as s2_pool,
            tc.tile_pool(name="osb", bufs=3) as osb_pool,
            tc.tile_pool(name="ps", bufs=4, space="PSUM") as ps_pool,
        ):
            wsel_sb = singles.tile([128, NT, 128], f16)
            nc.sync.dma_start(out=wsel_sb[:], in_=wsel_d[:, :, :])
            rep4_sb = singles.tile([NS, 128], f16)
            nc.sync.dma_start(out=rep4_sb[:], in_=rep4_d[:, :])
            md_sb = singles.tile([NS, TW], f16)
            nc.sync.dma_start(out=md_sb[:], in_=md_d[:, :])

            ADD_OP = mybir.AluOpType.add

            def load_batch(b):
                """Load one batch's x into a [128, 1600] x4d tile, and
                accumulate the three pre-scaled c-blocks of xv into the
                batch's scores tile s2 [32, 1600] via CCE add DMAs."""
                x4d = x4d_pool.tile([128, NQ * TW], f16, tag="x4d")
                nc.gpsimd.memset(x4d[96:128, :], 1.0)
                nc.sync.dma_start(out=x4d[0:96, :], in_=x_d[b, :, :])
                s2 = s2_pool.tile([NS, NQ * TW], f16, tag="s2")
                nc.sync.dma_start(out=s2[:], in_=xv_d[b, 0, :, :])
                nc.gpsimd.dma_start(out=s2[:], in_=xv_d[b, 1, :, :], accum_op=ADD_OP)
                nc.gpsimd.dma_start(out=s2[:], in_=xv_d[b, 2, :, :], accum_op=ADD_OP)
                return (x4d, s2)

            units = [(b, q) for b in range(BPC) for q in range(NQ)]
            nu = len(units)

            x4d_tiles = [None] * BPC
            x4d_tiles[0] = load_batch(0)
            if BPC > 1:
                x4d_tiles[1] = load_batch(1)

            def x4_view(ui):
                b, q = units[ui]
                return x4d_tiles[b][0][:, q * TW : (q + 1) * TW]

            def s2_view(ui):
                b, q = units[ui]
                return x4d_tiles[b][1][:, q * TW : (q + 1) * TW]

            def new_front():
                return ps_pool.tile([128, 2, BANK], f32, tag="ph2", name="front")

            def emit_exp(ui):
                e = sm_pool.tile([NS, TW], f16, tag="e")
                nc.scalar.activation(out=e[:], in_=s2_view(ui), func=EXP)
                return e

            def emit_zr(e):
                ev = e[:].rearrange("p (a b) -> p a b", b=N)
                z = sm_pool.tile([NS, FSUB], f16, tag="z")
                nc.vector.reduce_sum(out=z[:], in_=ev, axis=AX)
                r = sm_pool.tile([NS, FSUB], f16, tag="r")
                nc.vector.reciprocal(out=r[:], in_=z[:])
                return r

            def emit_em(e):
                em = sm_pool.tile([NS, TW], f16, tag="em")
                nc.gpsimd.tensor_tensor(out=em[:], in0=e[:], in1=md_sb[:], op=MULT)
                return em

            def emit_dd(em, r):
                dd = sm_pool.tile([NS, TW], f16, tag="dd")
                rr = r[:, :]
                r_bc = bass.AP(
                    tensor=rr.tensor,
                    offset=rr.offset,
                    ap=[rr.ap[0], [1, FSUB], [0, N]],
                )
                nc.gpsimd.tensor_tensor(out=dd[:], in0=em[:], in1=r_bc, op=MULT)
                return dd

            def emit_mm2_x4s(front, dd, ui):
                """pdd into front's bank 0, then x4s = x4 * pdd."""
                pdd = front[:, 0, 0:TW]
                nc.tensor.matmul(pdd, rep4_sb[:], dd[:], start=True, stop=True)
                x4s = x4s_pool.tile([128, TW], f16, tag="x4s")
                nc.vector.tensor_tensor(out=x4s[:], in0=x4_view(ui), in1=pdd, op=MULT)
                return x4s

            # ---- prologue: front-end chains for units 0 and 1
            with nc.allow_low_precision("fp16 softmax tolerates 1e-3"):
                def chain(ui):
                    e = emit_exp(ui)
                    r = emit_zr(e)
                    em = emit_em(e)
                    return emit_dd(em, r)

                dd_by_ui = {}
                dd_by_ui[0] = chain(0)
                if nu > 1:
                    dd_by_ui[1] = chain(1)
                front_cur = new_front()   # pdd_0
                x4s_cur = emit_mm2_x4s(front_cur, dd_by_ui.pop(0), 0)

                for ui, u in enumerate(units):
                    b, q = u
                    f0 = q * QF
                    have_next = ui + 1 < nu
                    x4s_u = x4s_cur
                    osb = osb_pool.tile([128, NT, TW], f16)

                    def pair(pr):
                        ph2 = ps_pool.tile(
                            [128, 2, BANK], f32, tag="ph2", name="ph2"
                        )
                        for i in (0, 1):
                            nc.tensor.matmul(
                                ph2[:, i, 0:TW],
                                wsel_sb[:, 2 * pr + i, :],
                                x4s_u[:, :],
                                start=True,
                                stop=True,
                            )
                        src = ph2[:, :, 0:TW]
                        dst = osb[:, 2 * pr : 2 * pr + 2, :]
                        dve = (pr % 2 == 0) if ui % 2 == 0 else (pr in (1, 3, 5))
                        if dve:
                            nc.vector.tensor_copy(dst, src)
                        else:
                            nc.scalar.copy(dst, src)

                    def store(hh):
                        osl = out_d[b, :, f0 : f0 + 1, :]
                        dmadst = bass.AP(
                            tensor=osl.tensor,
                            offset=osl.offset + hh * 8 * TW,
                            ap=[[FN, H], [16 * TW, 2], [1, 8 * TW]],
                        )
                        nc.sync.dma_start(
                            out=dmadst,
                            in_=osb[:, 8 * hh : 8 * (hh + 1), :],
                        )

                    have_far = ui + 2 < nu
                    if have_far:
                        e_n = emit_exp(ui + 2)
                    pair(0)
                    pair(1)
                    pair(2)
                    pair(3)
                    store(0)
                    if have_next:
                        front_nxt = new_front()  # pdd_{u+1}
                        x4s_cur = emit_mm2_x4s(front_nxt, dd_by_ui.pop(ui + 1), ui + 1)
                        front_cur = front_nxt
                    if have_far:
                        r_n = emit_zr(e_n)
                        em_n = emit_em(e_n)
                    pair(4)
                    pair(5)
                    if have_far:
                        dd_by_ui[ui + 2] = emit_dd(em_n, r_n)
                    pair(6)
                    pair(7)
                    store(1)
                    # prefetch the next batch near the end of each batch
                    if q == NQ - 1 and b + 2 < BPC and x4d_tiles[b + 2] is None:
                        x4d_tiles[b + 2] = load_batch(b + 2)
    nc.compile()
    return nc


def _get_nc():
    if "nc" not in _NC_CACHE:
        _NC_CACHE["nc"] = _build_nc()
    return _NC_CACHE["nc"]


def _make_in_maps(x, mask, W, bW, a1, a2, ab):
    x16 = np.asarray(x, np.float32).astype(np.float16)
    mask = np.asarray(mask, np.float32)
    W = np.asarray(W, np.float32)
    bW = np.asarray(bW, np.float32)
    a2 = np.asarray(a2, np.float32)

    # xr[b, 32c+fsub, (q, f', n)] = x[b, c, 512q + 16 fsub + f', n]
    xr = np.ascontiguousarray(
        x16.reshape(B, C, NQ, NS, FSUB, N)
        .transpose(0, 1, 3, 2, 4, 5)
        .reshape(B, C * NS, NQ * TW)
    )

    v = (W @ a2).astype(np.float32)                    # [C]
    md = np.diag(mask).astype(np.float16)              # [N]

    # xv[b, c, fsub, (q, f', n)] = v_c * x[b, c, 512q + 16 fsub + f', n]
    xf = np.asarray(x, np.float32)
    xv = np.ascontiguousarray(
        (v[None, :, None] * xf.reshape(B, C, F * N))
        .reshape(B, C, NQ, NS, FSUB, N)
        .transpose(0, 1, 3, 2, 4, 5)
        .reshape(B, C, NS, NQ * TW)
        .astype(np.float16)
    )

    # wsel[row = 32 c + fsub, tp, col = 2 k + jj]:
    #   delta[fsub == tp + 16 jj] * (W[c, k] if c < 3 else bW[k])
    # (column order (k, jj)-interleaved so the store DMA is affine)
    wsel = np.zeros((128, NT, 128), np.float16)
    cols = np.arange(H)
    W16 = W.astype(np.float16)
    bW16 = bW.astype(np.float16)
    for tp in range(NT):
        for jj in range(2):
            fsub = tp + 16 * jj
            for c in range(C):
                wsel[32 * c + fsub, tp, 2 * cols + jj] = W16[c]
            wsel[96 + fsub, tp, 2 * cols + jj] = bW16
    rep4 = np.zeros((NS, 128), np.float16)
    for blk in range(4):
        rep4[:, 32 * blk : 32 * (blk + 1)] = np.eye(NS, dtype=np.float16)
    md16 = np.tile(np.tile(md, FSUB)[None, :], (NS, 1)).astype(np.float16)

    in_maps = []
    for cix in range(NCORES):
        in_maps.append(
            {
                "xr": np.ascontiguousarray(xr[cix * BPC : (cix + 1) * BPC]),
                "wsel": wsel,
                "rep4": rep4,
                "xv": np.ascontiguousarray(xv[cix * BPC : (cix + 1) * BPC]),
                "md16": md16,
            }
        )
    return in_maps


def run(x, mask, W, bW, a1, a2, ab, **run_kwargs):
    from concourse.bass_utils import run_bass_kernel_spmd

    nc = _get_nc()
    in_maps = _make_in_maps(x, mask, W, bW, a1, a2, ab)
    res = run_bass_kernel_spmd(nc, in_maps, core_ids=list(range(NCORES)), **run_kwargs)
    out = np.concatenate(
        [res.results[i]["out"] for i in range(NCORES)], axis=0
    ).astype(np.float32)
    return out, res


def kernel(x, mask, W, bW, a1, a2, ab):
    out, _ = run(x, mask, W, bW, a1, a2, ab)
    return out


# revision 20
# speedup vs baseline: 1.0595x; 1.0366x over previous
"""Trainium2 Bass kernel for a GAT block.

Math (after algebraic simplification of the reference):
  h[b,f,n,k] = x[b,:,f,n] @ W[:,k] + bW[k]
  s2[b,f,n]  = h[b,f,n,:] @ a2 = v.x  (+c0 and s1/ab cancel inside softmax)
  d[b,f,n]   = softmax_n(s2)[n] * mask[n,n]
  out[b,k,f,n] = d[b,f,n] * h[b,f,n,k] = sum_c W[c,k] (x*d)[c,f,n] + bW[k] d[f,n]

Sharding: data-parallel over batch, 4 batches per core on 8 cores.

fp16 data path end to end (inputs converted and pre-transposed on host,
outputs converted back on host); all PE matmuls run at 1 cycle/row.

Per-batch input: xr [96, 4, 400] fp16 (row = 32c+fsub, cols (q, f', n),
frame = 16 fsub + f') loaded once into x4d [128, 1600] (rows 96:128
memset to 1.0 for the bias path; 3.2KB DMA descriptors).

Per (batch, 512-frame q-unit) on x4 = x4d[:, 400q:400q+400]:
  front-end: ps_s [32,400] = W2.T @ x4[0:96] (PE; scores in x4 layout),
    exp (ACT), reduce_sum + reciprocal (DVE, fp16), e*md and *1/z (GPS)
    -> dd [32,400] fp16
  MM2: pdd [128,400] = rep4.T @ dd (PE broadcast into 4 c-blocks)
  x4s [128,400] fp16 = x4 * pdd (DVE)
  8 matmul PAIRS into 2-bank psum tiles [128,2,512]; ONE eviction per
  pair (amortizes PSUM access latency), DVE pairs {0,2,4,6} / ACT
  {1,3,5,7}; 2 half stores per unit ([128,3200] fp16, 6.4KB
  descriptors), issued from the Sync sequencer.

PSUM is ONE ring of 4 two-bank slots (8 banks): per unit it carries
[pair0..pair6, front, pair7] where the front slot holds pdd of unit
u+1 (bank 0) and the scores of unit u+2 (bank 1).

Emission order is READINESS order per engine (engines dispatch
in-order; a not-yet-ready op at the queue head blocks later ready
ops).  Unit u's iteration interleaves: pairs 0-7 of u; exp/reduce/
recip/em/dd of u+1 staggered between pairs; and at the tail (between
pair 6 and 7) MM2+x4s of u+1 and MM1 of u+2 so the PE stream never
breaks at unit boundaries (PE p-state ramps to max only after ~3us of
continuous execution, doubling matmul speed).
"""

import sys

if "/opt/trn_rl_repo" not in sys.path:
    sys.path.insert(0, "/opt/trn_rl_repo")

import numpy as np

B, C, F, N, H = 32, 3, 2048, 25, 64
NCORES = 8
BPC = B // NCORES   # batches per core
QF = 512            # frames per q-unit
NQ = F // QF        # q-units per batch
FSUB = 16           # frames per fsub row
NS = QF // FSUB     # 32 fsub rows per q-unit
FN = F * N
TW = FSUB * N       # 400, columns per tile
NT = NS // 2        # 16 tiles (of 32 frames) per q-unit
NPAIR = NT // 2     # 8 psum pairs per q-unit
BANK = 512          # psum bank, f32 elems per partition

_NC_CACHE = {}


def _build_nc():
    import concourse.bass as bass
    import concourse.bacc as bacc
    import concourse.tile as tile
    from concourse import mybir

    f32 = mybir.dt.float32
    f16 = mybir.dt.float16
    MULT = mybir.AluOpType.mult
    AX = mybir.AxisListType.X
    EXP = mybir.ActivationFunctionType.Exp

    nc = bacc.Bacc()
    x_d = nc.declare_dram_parameter("xr", [BPC, 96, NQ * TW], f16, isOutput=False)
    wsel_d = nc.declare_dram_parameter("wsel", [128, NT, 128], f16, isOutput=False)
    rep4_d = nc.declare_dram_parameter("rep4", [NS, 128], f16, isOutput=False)
    xv_d = nc.declare_dram_parameter("xv", [BPC, C, NS, NQ * TW], f16, isOutput=False)
    md_d = nc.declare_dram_parameter("md16", [NS, TW], f16, isOutput=False)
    out_d = nc.declare_dram_parameter("out", [BPC, H, F, N], f16, isOutput=True)

    with tile.TileContext(nc) as tc:
        with (
            tc.tile_pool(name="singles", bufs=1) as singles,
            tc.tile_pool(name="x4d", bufs=3) as x4d_pool,
            tc.tile_pool(name="x4s", bufs=3) as x4s_pool,
            tc.tile_pool(name="sm", bufs=4) as sm_pool,
            tc.tile_pool(name="s2", bufs=3) as s2_pool,
            tc.tile_pool(name="osb", bufs=3) as osb_pool,
            tc.tile_pool(name="ps", bufs=4, space="PSUM") as ps_pool,
        ):
            ADD_OP = mybir.AluOpType.add

            def load_batch(b):
                """Load one batch's x into a [128, 1600] x4d tile, and
                accumulate the three pre-scaled c-blocks of xv into the
                batch's scores tile s2 [32, 1600] via CCE add DMAs."""
                x4d = x4d_pool.tile([128, NQ * TW], f16, tag="x4d")
                nc.gpsimd.memset(x4d[96:128, :], 1.0)
                nc.sync.dma_start(out=x4d[0:96, :], in_=x_d[b, :, :])
                s2 = s2_pool.tile([NS, NQ * TW], f16, tag="s2")
                nc.sync.dma_start(out=s2[:], in_=xv_d[b, 0, :, :])
                nc.gpsimd.dma_start(out=s2[:], in_=xv_d[b, 1, :, :], accum_op=ADD_OP)
                nc.gpsimd.dma_start(out=s2[:], in_=xv_d[b, 2, :, :], accum_op=ADD_OP)
                return (x4d, s2)

            units = [(b, q) for b in range(BPC) for q in range(NQ)]
            nu = len(units)

            x4d_tiles = [None] * BPC
            x4d_tiles[0] = load_batch(0)
            rep4_sb = singles.tile([NS, 128], f16)
            nc.sync.dma_start(out=rep4_sb[:], in_=rep4_d[:, :])
            md_sb = singles.tile([NS, TW], f16)
            nc.sync.dma_start(out=md_sb[:], in_=md_d[:, :])
            wsel_sb = singles.tile([128, NT, 128], f16)
            for wq in range(4):
                nc.sync.dma_start(
                    out=wsel_sb[32 * wq : 32 * (wq + 1), :, :],
                    in_=wsel_d[32 * wq : 32 * (wq + 1), :, :],
                )
            if BPC > 1:
                x4d_tiles[1] = load_batch(1)

            def x4_view(ui):
                b, q = units[ui]
                return x4d_tiles[b][0][:, q * TW : (q + 1) * TW]

            def s2_view(ui):
                b, q = units[ui]
                return x4d_tiles[b][1][:, q * TW : (q + 1) * TW]

            def new_front():
                return ps_pool.tile([128, 2, BANK], f32, tag="ph2", name="front")

            def emit_exp(ui):
                e = sm_pool.tile([NS, TW], f16, tag="e")
                nc.scalar.activation(out=e[:], in_=s2_view(ui), func=EXP)
                return e

            def emit_zr(e):
                ev = e[:].rearrange("p (a b) -> p a b", b=N)
                z = sm_pool.tile([NS, FSUB], f16, tag="z")
                nc.vector.reduce_sum(out=z[:], in_=ev, axis=AX)
                r = sm_pool.tile([NS, FSUB], f16, tag="r")
                nc.vector.reciprocal(out=r[:], in_=z[:])
                return r

            def emit_em(e):
                em = sm_pool.tile([NS, TW], f16, tag="em")
                nc.gpsimd.tensor_tensor(out=em[:], in0=e[:], in1=md_sb[:], op=MULT)
                return em

            def emit_dd(em, r):
                dd = sm_pool.tile([NS, TW], f16, tag="dd")
                rr = r[:, :]
                r_bc = bass.AP(
                    tensor=rr.tensor,
                    offset=rr.offset,
                    ap=[rr.ap[0], [1, FSUB], [0, N]],
                )
                nc.gpsimd.tensor_tensor(out=dd[:], in0=em[:], in1=r_bc, op=MULT)
                return dd

            def emit_mm2_x4s(front, dd, ui):
                """pdd into front's bank 0, then x4s = x4 * pdd."""
                pdd = front[:, 0, 0:TW]
                nc.tensor.matmul(pdd, rep4_sb[:], dd[:], start=True, stop=True)
                x4s = x4s_pool.tile([128, TW], f16, tag="x4s")
                nc.vector.tensor_tensor(out=x4s[:], in0=x4_view(ui), in1=pdd, op=MULT)
                return x4s

            # ---- prologue: front-end chains for units 0 and 1
            with nc.allow_low_precision("fp16 softmax tolerates 1e-3"):
                def chain(ui):
                    e = emit_exp(ui)
                    r = emit_zr(e)
                    em = emit_em(e)
                    return emit_dd(em, r)

                dd_by_ui = {}
                dd_by_ui[0] = chain(0)
                if nu > 1:
                    dd_by_ui[1] = chain(1)
                front_cur = new_front()   # pdd_0
                x4s_cur = emit_mm2_x4s(front_cur, dd_by_ui.pop(0), 0)

                for ui, u in enumerate(units):
                    b, q = u
                    f0 = q * QF
                    have_next = ui + 1 < nu
                    x4s_u = x4s_cur
                    osb = osb_pool.tile([128, NT, TW], f16)

                    def pair(pr):
                        ph2 = ps_pool.tile(
                            [128, 2, BANK], f32, tag="ph2", name="ph2"
                        )
                        for i in (0, 1):
                            nc.tensor.matmul(
                                ph2[:, i, 0:TW],
                                wsel_sb[:, 2 * pr + i, :],
                                x4s_u[:, :],
                                start=True,
                                stop=True,
                            )
                        src = ph2[:, :, 0:TW]
                        dst = osb[:, 2 * pr : 2 * pr + 2, :]
                        dve = (pr % 2 == 0) if ui % 2 == 0 else (pr in (1, 3, 5))
                        if dve:
                            nc.vector.tensor_copy(dst, src)
                        else:
                            nc.scalar.copy(dst, src)

                    def store(hh):
                        osl = out_d[b, :, f0 : f0 + 1, :]
                        if ui < nu - 1:
                            tail = [[1, 8 * TW]]
                        else:
                            # last unit: 1.6KB descriptors spread the
                            # drain across ~4x more DMA queues
                            tail = [[8 * TW // 4, 4], [1, 8 * TW // 4]]
                        dmadst = bass.AP(
                            tensor=osl.tensor,
                            offset=osl.offset + hh * 8 * TW,
                            ap=[[FN, H], [16 * TW, 2]] + tail,
                        )
                        nc.sync.dma_start(
                            out=dmadst,
                            in_=osb[:, 8 * hh : 8 * (hh + 1), :],
                        )

                    have_far = ui + 2 < nu
                    if have_far:
                        e_n = emit_exp(ui + 2)
                    pair(0)
                    pair(1)
                    pair(2)
                    pair(3)
                    store(0)
                    if have_next:
                        front_nxt = new_front()  # pdd_{u+1}
                        x4s_cur = emit_mm2_x4s(front_nxt, dd_by_ui.pop(ui + 1), ui + 1)
                        front_cur = front_nxt
                    if have_far:
                        r_n = emit_zr(e_n)
                        em_n = emit_em(e_n)
                    pair(4)
                    pair(5)
                    if have_far:
                        dd_by_ui[ui + 2] = emit_dd(em_n, r_n)
                    pair(6)
                    pair(7)
                    store(1)
                    # prefetch the next batch near the end of each batch
                    if q == NQ - 1 and b + 2 < BPC and x4d_tiles[b + 2] is None:
                        x4d_tiles[b + 2] = load_batch(b + 2)
    nc.compile()
    return nc


def _get_nc():
    if "nc" not in _NC_CACHE:
        _NC_CACHE["nc"] = _build_nc()
    return _NC_CACHE["nc"]


def _make_in_maps(x, mask, W, bW, a1, a2, ab):
    x16 = np.asarray(x, np.float32).astype(np.float16)
    mask = np.asarray(mask, np.float32)
    W = np.asarray(W, np.float32)
    bW = np.asarray(bW, np.float32)
    a2 = np.asarray(a2, np.float32)

    # xr[b, 32c+fsub, (q, f', n)] = x[b, c, 512q + 16 fsub + f', n]
    xr = np.ascontiguousarray(
        x16.reshape(B, C, NQ, NS, FSUB, N)
        .transpose(0, 1, 3, 2, 4, 5)
        .reshape(B, C * NS, NQ * TW)
    )

    v = (W @ a2).astype(np.float32)                    # [C]
    md = np.diag(mask).astype(np.float16)              # [N]

    # xv[b, c, fsub, (q, f', n)] = v_c * x[b, c, 512q + 16 fsub + f', n]
    xf = np.asarray(x, np.float32)
    xv = np.ascontiguousarray(
        (v[None, :, None] * xf.reshape(B, C, F * N))
        .reshape(B, C, NQ, NS, FSUB, N)
        .transpose(0, 1, 3, 2, 4, 5)
        .reshape(B, C, NS, NQ * TW)
        .astype(np.float16)
    )

    # wsel[row = 32 c + fsub, tp, col = 2 k + jj]:
    #   delta[fsub == tp + 16 jj] * (W[c, k] if c < 3 else bW[k])
    # (column order (k, jj)-interleaved so the store DMA is affine)
    wsel = np.zeros((128, NT, 128), np.float16)
    cols = np.arange(H)
    W16 = W.astype(np.float16)
    bW16 = bW.astype(np.float16)
    for tp in range(NT):
        for jj in range(2):
            fsub = tp + 16 * jj
            for c in range(C):
                wsel[32 * c + fsub, tp, 2 * cols + jj] = W16[c]
            wsel[96 + fsub, tp, 2 * cols + jj] = bW16
    rep4 = np.zeros((NS, 128), np.float16)
    for blk in range(4):
        rep4[:, 32 * blk : 32 * (blk + 1)] = np.eye(NS, dtype=np.float16)
    md16 = np.tile(np.tile(md, FSUB)[None, :], (NS, 1)).astype(np.float16)

    in_maps = []
    for cix in range(NCORES):
        in_maps.append(
            {
                "xr": np.ascontiguousarray(xr[cix * BPC : (cix + 1) * BPC]),
                "wsel": wsel,
                "rep4": rep4,
                "xv": np.ascontiguousarray(xv[cix * BPC : (cix + 1) * BPC]),
                "md16": md16,
            }
        )
    return in_maps


def run(x, mask, W, bW, a1, a2, ab, **run_kwargs):
    from concourse.bass_utils import run_bass_kernel_spmd

    nc = _get_nc()
    in_maps = _make_in_maps(x, mask, W, bW, a1, a2, ab)
    res = run_bass_kernel_spmd(nc, in_maps, core_ids=list(range(NCORES)), **run_kwargs)
    out = np.concatenate(
        [res.results[i]["out"] for i in range(NCORES)], axis=0
    ).astype(np.float32)
    return out, res


def kernel(x, mask, W, bW, a1, a2, ab):
    out, _ = run(x, mask, W, bW, a1, a2, ab)
    return out
